# revision 53
# baseline (speedup 1.0000x reference)
"""Trainium2 Bass kernel for MultiHeadDifferentialAttention.

Strategy: data-parallel over batch. B=8 batches map 1:1 onto the 8
NeuronCores; each core runs the full per-batch pipeline (QKV proj ->
differential attention -> LayerNorm -> output proj) with no collectives.
The host pre-lays-out inputs (x transposed per batch, weights reshaped
into partition-major tiles, gamma/beta/0.8 folded into Wp/bp) and
transposes the per-core [768, 1024] outputs back at the end.

Device pipeline per core:
  - qT/kT = (x @ Wq)^T per head in [2D=128, tok] fp16 layout: q1/q2 land
    on partitions 0-63 / 64-127, so the two K=64 score matmuls pack into
    disjoint PE row groups and run concurrently (measured: the pair
    costs one 512-col pass). They must target different PSUM banks -
    concurrent same-bank PE writes fault.
  - scores for TWO m-chunks land in one fp16 PSUM tile [128, 4, 512]
    (2 banks; quarters interleaved m0c1|m1c1|m0c2|m1c2 so each pair's
    two matmuls hit different banks) -> ONE strided exp per m-PAIR on
    ScalarE (scale fused) -> fp16 E tiles. Halving the exp instruction
    count keeps ScalarE's per-head time under the PE's so the PE (the
    roofline engine) is never starved.
  - v = x @ Wv into an augmented layout [tok, head, 128+1] whose last
    column is ones, so the attention-value matmul also produces the
    softmax denominator (column 128) for free.
  - AV: E tile is the stationary operand, rhs = [v_h | 1]; out[n, 0:128]
    is the unnormalized attention output, out[:, 128] the denominator.
    The two scores' accumulation chains share one PSUM bank (only the
    first matmul carries start=True - start clears the has_written bits
    bank-wide) and run un-interleaved so LDW/MM pairs pipeline.
  - combine a1 - lam*a2 and LN stats on VectorE (PSUM port), LN apply on
    GpSimd (SBUF-only), head transpose to the [1536, tok] layout via
    SBUF->SBUF DMA-xbar transposes (idle DMA engines; no PE, no PSUM).
    rsqrt = exp(-0.5*ln(var+eps)) on ScalarE: the activation-table patch
    below pins exp and ln to one table set, so no reloads.
  - final projection from o_lnT with fp16 weights, bias folded, output
    DMA'd in split chunks. Output is F^T [768, 1024].

Scheduling (perfetto-driven; the PE executes its queue IN ORDER, and
every idle gap drops the DVFS clock for the next ~1-3us, so the one
objective is a dense PE queue whose next instruction is always ready):
  - wq0/wk0 are DMA'd first (the PE's first instructions need them),
    then x, then wq1/wk1, then wv; Wp prefetches during head 9.
  - head loop is software-pipelined: per head h emits
      scores(h,r0) | qk-half0(h+1) | AV(h-1,r1)+tail(h-1) |
      scores(h,r1) | qk-half1(h+1) | AV(h,r0)
    so ~6.2us of independent PE work sits between a strip's scores and
    the next strip's (which gates on that strip's exp via sp reuse).
  - the V projection is nested into head 0's slot (its PSUM pool is
    scoped there, closed before the AV pool opens; 8-bank budget).
"""

import numpy as np

B, N, C, H = 8, 1024, 768, 12
D = C // H  # 64
TD = 2 * D  # 128
LAMBDA_INIT = 0.8 - 0.6 * np.exp(-0.3 * (1 - 1))  # 0.2
OUT_SCALE = 1.0 - LAMBDA_INIT  # 0.8
EPS = 1e-5
SCALE = float(D) ** -0.5  # 1/8

_BUILD_CACHE = {}
LAST_EXEC_NS = None


def _patch_act_tables(mybir, bacc):
    """Pin Exp and Ln to natural_log_exp_and_others so interleaving them
    never reloads the ScalarE spline tables."""
    from concourse import hw_specs

    orig = hw_specs.get_activation_tables
    if getattr(bacc.get_activation_tables, "_nlx_pinned", False):
        return

    def patched(arch):
        tables = orig(arch)
        exp = mybir.ActivationFunctionType.Exp
        ln = mybir.ActivationFunctionType.Ln
        for name, funcs in tables.items():
            if name != "natural_log_exp_and_others":
                funcs.discard(exp)
                funcs.discard(ln)
        return tables

    patched._nlx_pinned = True
    bacc.get_activation_tables = patched


def _build(lam: float, dbg: bool = False):
    from contextlib import ExitStack

    import concourse.bass as bass  # noqa: F401
    import concourse.mybir as mybir
    import concourse.tile as tile
    from concourse import bacc

    _patch_act_tables(mybir, bacc)

    f32 = mybir.dt.float32
    f16 = mybir.dt.float16
    AF = mybir.ActivationFunctionType
    OP = mybir.AluOpType

    nc = bacc.Bacc(None, target_bir_lowering=False, debug=False)

    XT = nc.declare_dram_parameter("xT", [128, 6, 1024], f16, isOutput=False)
    WQR = nc.declare_dram_parameter("WqR", [12, 128, 6, 128], f16, isOutput=False)
    WKR = nc.declare_dram_parameter("WkR", [12, 128, 6, 128], f16, isOutput=False)
    WVR = nc.declare_dram_parameter("WvR", [128, 6, 1536], f16, isOutput=False)
    WPR = nc.declare_dram_parameter("WpR", [12, 128, 768], f16, isOutput=False)
    BPP = nc.declare_dram_parameter("bpp", [128, 6], f32, isOutput=False)
    OUT = nc.declare_dram_parameter("outT", [128, 6, 1024], f32, isOutput=True)

    with tile.TileContext(nc) as tc, ExitStack() as top:
        persist = top.enter_context(tc.tile_pool(name="persist", bufs=1))
        o_ln = persist.tile([128, 8, 12, 128], f16)
        o_lnT = persist.tile([128, 12, 1024], f16)
        stats_sb = persist.tile([128, 12, 8, 2], f32)
        sdbuf = persist.tile([128, 12, 8], f32)
        rsbuf = persist.tile([128, 12, 8], f32)
        bpp_sb = persist.tile([128, 6], f32)
        eps_sb = persist.tile([128, 1], f32)

        longA = top.enter_context(tc.tile_pool(name="longA", bufs=1))
        xT_all = longA.tile([128, 6, 1024], f16, name="xT_all")
        xTk = [xT_all[:, k] for k in range(6)]
        v_aug = longA.tile([128, 8, 12, 129], f16)

        # wps/wv outlive the attention pools (pool creation order must be
        # a strict stack; DMA issue order is set by dma_start placement)
        wpsp = top.enter_context(tc.tile_pool(name="wps", bufs=12))
        wpks = []
        wvp = top.enter_context(tc.tile_pool(name="wv", bufs=1))
        wv_all = wvp.tile([128, 6, 1536], f16, name="wv_all")
        wvk = [wv_all[:, k] for k in range(6)]

        attn = ExitStack()
        wqkp = attn.enter_context(tc.tile_pool(name="wqk", bufs=3))
        qkp = attn.enter_context(tc.tile_pool(name="qk", bufs=3))
        qkps = attn.enter_context(tc.tile_pool(name="qkps", bufs=2, space="PSUM"))

        # ---- head-0/1 q/k weights first: the PE's first queued
        # instructions are head 0's q/k projection matmuls.
        def emit_qk_dma(h):
            wqh = wqkp.tile([128, 6, 128], f16, tag="wq", name=f"wqh{h}")
            wkh = wqkp.tile([128, 6, 128], f16, tag="wk", name=f"wkh{h}")
            nc.sync.dma_start(out=wqh[:], in_=WQR[h])
            nc.sync.dma_start(out=wkh[:], in_=WKR[h])
            qh = qkp.tile([128, 1024], f16, tag="q", name=f"qh{h}")
            kh = qkp.tile([128, 1024], f16, tag="k", name=f"kh{h}")
            return wqh, wkh, qh, kh

        st0 = emit_qk_dma(0)
        nc.sync.dma_start(out=bpp_sb[:], in_=BPP[:])
        # few issues: each dma_start costs ~594ns of Sync-engine
        # descriptor generation (measured). x splits in two so head 0's
        # projection can start on the first half sooner.
        nc.sync.dma_start(out=xT_all[:, 0:3], in_=XT[:, 0:3])
        nc.sync.dma_start(out=xT_all[:, 3:6], in_=XT[:, 3:6])
        st1 = emit_qk_dma(1)
        nc.sync.dma_start(out=wv_all[:], in_=WVR[:])

        nc.vector.memset(v_aug[:, :, :, 128:129], 1.0)
        nc.vector.memset(eps_sb[:], EPS)

        # ---- PE warm-up ----
        # The PE clock ramps with sustained activity (idle drops it to
        # ~1.2GHz and measured recovery takes several us). While the input
        # DMAs land, run throwaway matmuls on a zeroed scratch tile so the
        # first real matmuls start at full clock.
        with (
            tc.tile_pool(name="warm", bufs=1) as warmp,
            tc.tile_pool(name="warmps", bufs=1, space="PSUM") as warmps,
        ):
            wsrc = warmp.tile([128, 512], f16, name="warm_src")
            nc.vector.memset(wsrc[:], 0.0)
            wps = warmps.tile([128, 512], f32, name="warm_ps")
            for _ in range(16):
                nc.tensor.matmul(
                    wps[:], wsrc[:, 0:128], wsrc[:],
                    start=True, stop=True, skip_group_check=True,
                )

        def qk_half(wt, dst, xo):
            """One 512-token half of a q/k projection (6 MMs + copy)."""
            ps = qkps.tile([128, 512], f32, tag="qk", name="ps_qk")
            for k in range(6):
                nc.tensor.matmul(
                    ps[:], wt[:, k, :], xTk[k][:, xo : xo + 512],
                    start=(k == 0), stop=(k == 5),
                )
            nc.vector.tensor_copy(dst[:, xo : xo + 512], ps[:])

        def emit_qk_mms(state, half):
            wqh, wkh, qh, kh = state
            qk_half(wqh, qh, half * 512)
            qk_half(wkh, kh, half * 512)

        # head 0's projection, both halves, right away
        emit_qk_mms(st0, 0)
        emit_qk_mms(st0, 1)

        ep = attn.enter_context(tc.tile_pool(name="estrip", bufs=3))
        fin = attn.enter_context(tc.tile_pool(name="fin", bufs=4))
        tstage = attn.enter_context(tc.tile_pool(name="tstage", bufs=2))
        spool = attn.enter_context(tc.tile_pool(name="spool", bufs=2, space="PSUM"))
        avps = None  # opened after the V projection's pool closes

        def emit_score_m(qh, kh, r, e12, m):
            """Scores+exp for one m-chunk of a 512-token strip. The two
            chain matmuls use disjoint PE row groups and disjoint PSUM
            banks, so the pair streams as one 512-col pass; one exp
            covers both chains."""
            nsl = slice(r * 512, (r + 1) * 512)
            msl = slice(m * 128, (m + 1) * 128)
            sp = spool.tile([128, 2, 512], f32, tag="s")
            nc.tensor.matmul(
                sp[:, 0, :], kh[0:64, msl], qh[0:64, nsl],
                start=True, stop=True,
            )
            nc.tensor.matmul(
                sp[:, 1, :], kh[64:128, msl], qh[64:128, nsl],
                start=True, stop=True,
            )
            nc.scalar.activation(
                e12[:, m, :].rearrange("p (a b) -> p a b", a=2),
                sp[:],
                AF.Exp,
                scale=SCALE,
            )

        def do_av_chunk(h, r, e12, c2):
                jn = r * 4 + c2
                o = avps.tile([128, 258], f32, tag="o", name="o_av")
                # Both accumulation chains share one PSUM bank. start=True
                # clears has_written bank-wide, so only the very first
                # matmul may set it; the second chain's first write still
                # overwrites because its bits are already clear. Chains
                # un-interleaved so consecutive LDW/MM pairs pipeline.
                for m in range(8):
                    nc.tensor.matmul(
                        o[:, 0:129],
                        e12[:, m, c2 * 128 : (c2 + 1) * 128],
                        v_aug[:, m, h, :],
                        start=(m == 0),
                        stop=(m == 7),
                        skip_group_check=True,
                    )
                for m in range(8):
                    nc.tensor.matmul(
                        o[:, 129:258],
                        e12[:, m, 512 + c2 * 128 : 512 + (c2 + 1) * 128],
                        v_aug[:, m, h, :],
                        start=False,
                        stop=(m == 7),
                        skip_group_check=True,
                    )
                # combine + LN stats (VectorE: PSUM port). One strided
                # reciprocal covers both denominators (cols 128 and 257).
                r12 = fin.tile([128, 2], f32, tag="r12")
                nc.vector.reciprocal(
                    r12[:], o[:].rearrange("p (c x) -> p c x", c=2)[:, :, 128:129]
                )
                t2 = fin.tile([128, 128], f32, tag="t2")
                nc.vector.tensor_scalar(
                    t2[:], o[:, 129:257], r12[:, 1:2], float(lam),
                    op0=OP.mult, op1=OP.mult,
                )
                nc.vector.scalar_tensor_tensor(
                    o_ln[:, jn, h, :],
                    o[:, 0:128],
                    r12[:, 0:1],
                    t2[:],
                    op0=OP.mult,
                    op1=OP.subtract,
                )
                st6 = fin.tile([128, 6], f32, tag="st6")
                nc.vector.bn_stats(st6[:], o_ln[:, jn, h, :])
                nc.vector.bn_aggr(stats_sb[:, h, jn, :], st6[:])

        def do_av(h, r, e12):
            for c2 in range(4):
                do_av_chunk(h, r, e12, c2)

        def head_tail(h):
            """rsqrt, LN apply, DMA-xbar transposes into a staging tile,
            then ONE GpSimd copy into o_lnT. The copy launders the 8
            DMA-queue semaphores into a single engine semaphore: without
            it every final-projection matmul re-waits on the DMA sems
            (~160ns each, measured). It runs on GpSimd because that queue
            is otherwise empty - it can block on the Sync engine's slow
            xbar transposes (~1.2us each, measured) without stalling
            anything."""
            # rs = exp(-0.5 * ln(var + eps)); Exp and Ln share one pinned
            # table set, so no reload happens here.
            nc.scalar.activation(
                sdbuf[:, h, :], stats_sb[:, h, :, 1], AF.Ln, bias=eps_sb[:],
            )
            nc.scalar.activation(
                rsbuf[:, h, :], sdbuf[:, h, :], AF.Exp, scale=-0.5,
            )
            last = h == 11
            stg = None if last else tstage.tile([128, 1024], f16, tag="tstage")
            for jn in range(8):
                nc.vector.tensor_scalar(
                    o_ln[:, jn, h, :],
                    o_ln[:, jn, h, :],
                    stats_sb[:, h, jn, 0:1],
                    rsbuf[:, h, jn : jn + 1],
                    op0=OP.subtract,
                    op1=OP.mult,
                )
                # head 11 is on the final-projection critical path: write
                # o_lnT directly, skipping the staging+copy latency (its
                # k=11 matmuls eat the small per-MM DMA-sem wait instead).
                dst = o_lnT[:, h, :] if last else stg[:]
                nc.sync.dma_start(
                    out=dst[:, jn * 128 : (jn + 1) * 128],
                    in_=o_ln[:, jn, h, :],
                    transpose=True,
                )
            if not last:
                nc.gpsimd.tensor_copy(o_lnT[:, h, :], stg[:])

        # ---- software-pipelined head loop ----
        # Per head h (steady state), the PE emission order is
        #   block A (strip r0):
        #     sc mp0, sc mp1, qk(h+1) q-half0, sc mp2, qk k-half0, sc mp3,
        #     AV(h-1, r1) c0..c3, tail(h-1)
        #   block B (strip r1):
        #     sc mp0, AV(h,r0) c0, sc mp1, AV c1, sc mp2, AV c2, sc mp3,
        #     AV c3, qk q-half1, qk k-half1
        # chosen so that each score pair's gate (the exp freeing its PSUM
        # tile, 2 tiles back) has completed by the time the PE reaches it.
        # h==0 nests the V projection (and its scoped PSUM pool) in place
        # of the AV work it doesn't have yet.
        states = {0: st0, 1: st1}
        pend = None  # (h, e12 of strip r1) awaiting AV + tail
        for h in range(12):
            qh, kh = states[h][2], states[h][3]
            stn = None
            if h + 1 < 12:
                stn = states[h + 1]
            if h + 2 < 12:
                states[h + 2] = emit_qk_dma(h + 2)
            states.pop(h - 1, None)
            if h == 9:
                # prefetch final-proj weights over the last heads' window
                for kk in range(12):
                    wpk = wpsp.tile([128, 768], f16, tag="wp", name=f"wpk{kk}")
                    nc.sync.dma_start(out=wpk[:], in_=WPR[kk])
                    wpks.append(wpk)

            # ---- block A (strip r0) ----
            e12_r0 = ep.tile([128, 8, 1024], f16, tag="e")
            for m in range(4):
                emit_score_m(qh, kh, 0, e12_r0, m)
            if stn is not None:
                qk_half(stn[0], stn[2], 0)
            for m in range(4, 8):
                emit_score_m(qh, kh, 0, e12_r0, m)
            if stn is not None:
                qk_half(stn[1], stn[3], 0)

            if h == 0:
                # ---- V projection, nested in head 0's slot ----
                with tc.tile_pool(name="vps", bufs=2, space="PSUM") as vps:
                    for t in range(8):
                        for cr in range(3):
                            ps = vps.tile([128, 512], f32, tag="v", name="vps")
                            for k in range(6):
                                nc.tensor.matmul(
                                    ps[:],
                                    xTk[k][:, t * 128 : (t + 1) * 128],
                                    wvk[k][:, cr * 512 : (cr + 1) * 512],
                                    start=(k == 0),
                                    stop=(k == 5),
                                )
                            nc.vector.tensor_copy(
                                v_aug[:, t, 4 * cr : 4 * cr + 4, 0:128],
                                ps[:].rearrange("p (h c) -> p h c", c=128),
                            )
                avps = attn.enter_context(
                    tc.tile_pool(name="avps", bufs=2, space="PSUM")
                )
            else:
                do_av(pend[0], 1, pend[1])
                head_tail(pend[0])

            # ---- block B (strip r1) ----
            e12_r1 = ep.tile([128, 8, 1024], f16, tag="e")
            for c2 in range(4):
                emit_score_m(qh, kh, 1, e12_r1, 2 * c2)
                emit_score_m(qh, kh, 1, e12_r1, 2 * c2 + 1)
                do_av_chunk(h, 0, e12_r0, c2)
            if stn is not None:
                qk_half(stn[0], stn[2], 512)
                qk_half(stn[1], stn[3], 512)
            pend = (h, e12_r1)

        do_av(pend[0], 1, pend[1])
        head_tail(pend[0])
        attn.close()  # frees qkps/spool/avps PSUM + qk/e12 SBUF

        # ---- Phase 3: final projection ----
        # mc-outer / k-inner: each 128-wide output-channel block finishes
        # (bias + DMA) while the next block computes, so only the last
        # block's drain is exposed. Stationary is reused across the two
        # token halves so walrus elides every second weight load.
        with (
            tc.tile_pool(name="tail", bufs=1) as tailp,
            tc.tile_pool(name="fps", bufs=3, space="PSUM") as fps,
        ):
            fout = tailp.tile([128, 6, 1024], f32)

            def mc_mms(mc, ks, fs):
                for k in ks:
                    wpk = wpks[k]
                    for nr2 in range(2):
                        nc.tensor.matmul(
                            fs[nr2][:],
                            wpk[:, mc * 128 : (mc + 1) * 128],
                            o_lnT[:, k, nr2 * 512 : (nr2 + 1) * 512],
                            start=(k == 0),
                            stop=(k == 11),
                        )

            def mc_drain(mc, fs):
                for nr2 in range(2):
                    nsl2 = slice(nr2 * 512, (nr2 + 1) * 512)
                    nc.vector.tensor_scalar(
                        fout[:, mc, nsl2],
                        fs[nr2][:],
                        bpp_sb[:, mc : mc + 1],
                        None,
                        op0=OP.add,
                    )
                # one issue per mc: descriptor generation on Sync is the
                # tail bottleneck, and one dma_start's descriptors already
                # spread over all 16 queues. The last block splits in two
                # so its final bytes ride partly-drained queues.
                if mc == 5:
                    nc.sync.dma_start(
                        out=OUT[:, mc, 0:512], in_=fout[:, mc, 0:512]
                    )
                    nc.sync.dma_start(
                        out=OUT[:, mc, 512:1024], in_=fout[:, mc, 512:1024]
                    )
                else:
                    nc.sync.dma_start(out=OUT[:, mc], in_=fout[:, mc])

            def fs_tiles():
                return [
                    fps.tile([128, 512], f32, tag=f"f{nr2}", name=f"fps{nr2}")
                    for nr2 in range(2)
                ]

            # The first three blocks defer their k=11 contribution: head
            # 11's transposes are still landing when phase 3 starts, and
            # ~66 k<11 matmuls are enough PE work to cover that latency
            # without a (clock-dropping) stall.
            fs012 = [fs_tiles() for _ in range(3)]
            for mc in range(3):
                mc_mms(mc, range(11), fs012[mc])
            for mc in range(3):
                mc_mms(mc, [11], fs012[mc])
                mc_drain(mc, fs012[mc])
            for mc in range(3, 5):
                fs = fs_tiles()
                mc_mms(mc, range(12), fs)
                mc_drain(mc, fs)
            # last block: finish the first token-half early so its bias+DMA
            # drain overlaps the second half's matmuls instead of the tail
            fs = fs_tiles()
            for k in range(12):
                nc.tensor.matmul(
                    fs[0][:], wpks[k][:, 5 * 128 : 6 * 128],
                    o_lnT[:, k, 0:512], start=(k == 0), stop=(k == 11),
                )
            nc.vector.tensor_scalar(
                fout[:, 5, 0:512], fs[0][:], bpp_sb[:, 5:6], None, op0=OP.add,
            )
            nc.sync.dma_start(out=OUT[:, 5, 0:512], in_=fout[:, 5, 0:512])
            for k in range(12):
                nc.tensor.matmul(
                    fs[1][:], wpks[k][:, 5 * 128 : 6 * 128],
                    o_lnT[:, k, 512:1024], start=(k == 0), stop=(k == 11),
                )
            nc.vector.tensor_scalar(
                fout[:, 5, 512:1024], fs[1][:], bpp_sb[:, 5:6], None,
                op0=OP.add,
            )
            nc.sync.dma_start(out=OUT[:, 5, 512:1024], in_=fout[:, 5, 512:1024])

    nc.compile()
    return nc


def _host_prep(x, Wq, Wk, Wv, gamma, beta, Wp, bp):
    x = np.ascontiguousarray(np.asarray(x, np.float32))
    Wq = np.asarray(Wq, np.float32)
    Wk = np.asarray(Wk, np.float32)
    Wv = np.asarray(Wv, np.float32)
    Wp = np.asarray(Wp, np.float32)
    bp = np.asarray(bp, np.float32)
    gamma = np.asarray(gamma, np.float32)
    beta = np.asarray(beta, np.float32)

    # xT per batch: [128, 6, 1024] with [p, k, n] = x[b, n, k*128+p]
    xTr = np.ascontiguousarray(
        x.transpose(0, 2, 1).reshape(B, 6, 128, N).transpose(0, 2, 1, 3)
    ).astype(np.float16)

    # W[qk]R: [12, 128, 6, 128] with [h, p, k, c] = W[k*128+p, h*128+c]
    def wqk_r(W):
        return np.ascontiguousarray(
            W.reshape(6, 128, 12, 128).transpose(2, 1, 0, 3)
        )

    WqR = wqk_r(Wq).astype(np.float16)
    WkR = wqk_r(Wk).astype(np.float16)
    # WvR: [128, 6, 1536] with [p, k, c] = Wv[k*128+p, c]
    WvR = np.ascontiguousarray(
        Wv.reshape(6, 128, 2 * C).transpose(1, 0, 2)
    ).astype(np.float16)
    # Fold gamma and the (1 - lambda_init) scale into Wp; beta into the bias.
    gfull = np.tile(gamma, H)  # [1536]
    Wpg = Wp * (OUT_SCALE * gfull)[:, None]
    bpp = bp + OUT_SCALE * (np.tile(beta, H) @ Wp)
    WpR = np.ascontiguousarray(Wpg.reshape(12, 128, C)).astype(np.float16)
    bppR = np.ascontiguousarray(bpp.reshape(6, 128).T)  # [128, 6]
    return xTr, WqR, WkR, WvR, WpR, bppR


def kernel(x, Wq, Wk, Wv, lam, gamma, beta, Wp, bp):
    global LAST_EXEC_NS
    import os

    from concourse.bass_utils import run_bass_kernel_spmd

    lam_f = float(np.asarray(lam))
    xTr, WqR, WkR, WvR, WpR, bppR = _host_prep(
        x, Wq, Wk, Wv, gamma, beta, Wp, bp
    )

    key = lam_f
    if key not in _BUILD_CACHE:
        _BUILD_CACHE[key] = _build(lam_f)
    nc = _BUILD_CACHE[key]

    in_maps = [
        {
            "xT": xTr[b],
            "WqR": WqR,
            "WkR": WkR,
            "WvR": WvR,
            "WpR": WpR,
            "bpp": bppR,
        }
        for b in range(B)
    ]

    trace = bool(os.environ.get("BASS_KERNEL_TRACE"))
    if trace:
        from concourse import bass_utils as _bu

        _bu.upload_artifacts = lambda tmpdir: "local://" + tmpdir
    res = run_bass_kernel_spmd(
        nc, in_maps, list(range(B)), trace=trace,
        **({"trace_cores": list(range(B))} if trace else {}),
    )
    LAST_EXEC_NS = res.exec_time_ns

    out = np.empty((B, N, C), np.float32)
    for b in range(B):
        outT = res.results[b]["outT"]  # [128, 6, 1024]
        out[b] = outT.transpose(2, 1, 0).reshape(N, C)
    return out


# revision 54
# speedup vs baseline: 1.0067x; 1.0067x over previous
"""Trainium2 Bass kernel for MultiHeadDifferentialAttention.

Strategy: data-parallel over batch. B=8 batches map 1:1 onto the 8
NeuronCores; each core runs the full per-batch pipeline (QKV proj ->
differential attention -> LayerNorm -> output proj) with no collectives.
The host pre-lays-out inputs (x transposed per batch, weights reshaped
into partition-major tiles, gamma/beta/0.8 folded into Wp/bp) and
transposes the per-core [768, 1024] outputs back at the end.

Device pipeline per core:
  - qT/kT = (x @ Wq)^T per head in [2D=128, tok] fp16 layout: q1/q2 land
    on partitions 0-63 / 64-127, so the two K=64 score matmuls pack into
    disjoint PE row groups and run concurrently (measured: the pair
    costs one 512-col pass). They must target different PSUM banks -
    concurrent same-bank PE writes fault.
  - scores for TWO m-chunks land in one fp16 PSUM tile [128, 4, 512]
    (2 banks; quarters interleaved m0c1|m1c1|m0c2|m1c2 so each pair's
    two matmuls hit different banks) -> ONE strided exp per m-PAIR on
    ScalarE (scale fused) -> fp16 E tiles. Halving the exp instruction
    count keeps ScalarE's per-head time under the PE's so the PE (the
    roofline engine) is never starved.
  - v = x @ Wv into an augmented layout [tok, head, 128+1] whose last
    column is ones, so the attention-value matmul also produces the
    softmax denominator (column 128) for free.
  - AV: E tile is the stationary operand, rhs = [v_h | 1]; out[n, 0:128]
    is the unnormalized attention output, out[:, 128] the denominator.
    The two scores' accumulation chains share one PSUM bank (only the
    first matmul carries start=True - start clears the has_written bits
    bank-wide) and run un-interleaved so LDW/MM pairs pipeline.
  - combine a1 - lam*a2 and LN stats on VectorE (PSUM port), LN apply on
    GpSimd (SBUF-only), head transpose to the [1536, tok] layout via
    SBUF->SBUF DMA-xbar transposes (idle DMA engines; no PE, no PSUM).
    rsqrt = exp(-0.5*ln(var+eps)) on ScalarE: the activation-table patch
    below pins exp and ln to one table set, so no reloads.
  - final projection from o_lnT with fp16 weights, bias folded, output
    DMA'd in split chunks. Output is F^T [768, 1024].

Scheduling (perfetto-driven; the PE executes its queue IN ORDER, and
every idle gap drops the DVFS clock for the next ~1-3us, so the one
objective is a dense PE queue whose next instruction is always ready):
  - wq0/wk0 are DMA'd first (the PE's first instructions need them),
    then x, then wq1/wk1, then wv; Wp prefetches during head 9.
  - head loop is software-pipelined: per head h emits
      scores(h,r0) | qk-half0(h+1) | AV(h-1,r1)+tail(h-1) |
      scores(h,r1) | qk-half1(h+1) | AV(h,r0)
    so ~6.2us of independent PE work sits between a strip's scores and
    the next strip's (which gates on that strip's exp via sp reuse).
  - the V projection is nested into head 0's slot (its PSUM pool is
    scoped there, closed before the AV pool opens; 8-bank budget).
"""

import numpy as np

B, N, C, H = 8, 1024, 768, 12
D = C // H  # 64
TD = 2 * D  # 128
LAMBDA_INIT = 0.8 - 0.6 * np.exp(-0.3 * (1 - 1))  # 0.2
OUT_SCALE = 1.0 - LAMBDA_INIT  # 0.8
EPS = 1e-5
SCALE = float(D) ** -0.5  # 1/8

_BUILD_CACHE = {}
LAST_EXEC_NS = None


def _patch_act_tables(mybir, bacc):
    """Pin Exp and Ln to natural_log_exp_and_others so interleaving them
    never reloads the ScalarE spline tables."""
    from concourse import hw_specs

    orig = hw_specs.get_activation_tables
    if getattr(bacc.get_activation_tables, "_nlx_pinned", False):
        return

    def patched(arch):
        tables = orig(arch)
        exp = mybir.ActivationFunctionType.Exp
        ln = mybir.ActivationFunctionType.Ln
        for name, funcs in tables.items():
            if name != "natural_log_exp_and_others":
                funcs.discard(exp)
                funcs.discard(ln)
        return tables

    patched._nlx_pinned = True
    bacc.get_activation_tables = patched


def _build(lam: float, dbg: bool = False):
    from contextlib import ExitStack

    import concourse.bass as bass  # noqa: F401
    import concourse.mybir as mybir
    import concourse.tile as tile
    from concourse import bacc

    _patch_act_tables(mybir, bacc)

    f32 = mybir.dt.float32
    f16 = mybir.dt.float16
    AF = mybir.ActivationFunctionType
    OP = mybir.AluOpType

    nc = bacc.Bacc(None, target_bir_lowering=False, debug=False)

    XT = nc.declare_dram_parameter("xT", [128, 6, 1024], f16, isOutput=False)
    WQR = nc.declare_dram_parameter("WqR", [12, 128, 6, 128], f16, isOutput=False)
    WKR = nc.declare_dram_parameter("WkR", [12, 128, 6, 128], f16, isOutput=False)
    WVR = nc.declare_dram_parameter("WvR", [128, 6, 1536], f16, isOutput=False)
    WPR = nc.declare_dram_parameter("WpR", [12, 128, 768], f16, isOutput=False)
    BPP = nc.declare_dram_parameter("bpp", [128, 6], f32, isOutput=False)
    OUT = nc.declare_dram_parameter("outT", [128, 6, 1024], f32, isOutput=True)

    with tile.TileContext(nc) as tc, ExitStack() as top:
        persist = top.enter_context(tc.tile_pool(name="persist", bufs=1))
        o_ln = persist.tile([128, 8, 12, 128], f16)
        o_lnT = persist.tile([128, 12, 1024], f16)
        stats_sb = persist.tile([128, 12, 8, 2], f32)
        sdbuf = persist.tile([128, 12, 8], f32)
        rsbuf = persist.tile([128, 12, 8], f32)
        bpp_sb = persist.tile([128, 6], f32)
        eps_sb = persist.tile([128, 1], f32)

        longA = top.enter_context(tc.tile_pool(name="longA", bufs=1))
        xT_all = longA.tile([128, 6, 1024], f16, name="xT_all")
        xTk = [xT_all[:, k] for k in range(6)]
        v_aug = longA.tile([128, 8, 12, 129], f16)

        # wps/wv outlive the attention pools (pool creation order must be
        # a strict stack; DMA issue order is set by dma_start placement)
        wpsp = top.enter_context(tc.tile_pool(name="wps", bufs=12))
        wpks = []
        wvp = top.enter_context(tc.tile_pool(name="wv", bufs=1))
        wv_all = wvp.tile([128, 6, 1536], f16, name="wv_all")
        wvk = [wv_all[:, k] for k in range(6)]

        attn = ExitStack()
        wqkp = attn.enter_context(tc.tile_pool(name="wqk", bufs=3))
        qkp = attn.enter_context(tc.tile_pool(name="qk", bufs=3))
        qkps = attn.enter_context(tc.tile_pool(name="qkps", bufs=2, space="PSUM"))

        # ---- head-0/1 q/k weights first: the PE's first queued
        # instructions are head 0's q/k projection matmuls.
        def emit_qk_dma(h):
            wqh = wqkp.tile([128, 6, 128], f16, tag="wq", name=f"wqh{h}")
            wkh = wqkp.tile([128, 6, 128], f16, tag="wk", name=f"wkh{h}")
            nc.sync.dma_start(out=wqh[:], in_=WQR[h])
            nc.sync.dma_start(out=wkh[:], in_=WKR[h])
            qh = qkp.tile([128, 1024], f16, tag="q", name=f"qh{h}")
            kh = qkp.tile([128, 1024], f16, tag="k", name=f"kh{h}")
            return wqh, wkh, qh, kh

        st0 = emit_qk_dma(0)
        nc.sync.dma_start(out=bpp_sb[:], in_=BPP[:])
        # few issues: each dma_start costs ~594ns of Sync-engine
        # descriptor generation (measured). x splits in two so head 0's
        # projection can start on the first half sooner.
        nc.sync.dma_start(out=xT_all[:, 0:3], in_=XT[:, 0:3])
        nc.sync.dma_start(out=xT_all[:, 3:6], in_=XT[:, 3:6])
        st1 = emit_qk_dma(1)
        nc.sync.dma_start(out=wv_all[:], in_=WVR[:])

        nc.vector.memset(v_aug[:, :, :, 128:129], 1.0)
        nc.vector.memset(eps_sb[:], EPS)

        # ---- PE warm-up ----
        # The PE clock ramps with sustained activity (idle drops it to
        # ~1.2GHz and measured recovery takes several us). While the input
        # DMAs land, run throwaway matmuls on a zeroed scratch tile so the
        # first real matmuls start at full clock.
        with (
            tc.tile_pool(name="warm", bufs=1) as warmp,
            tc.tile_pool(name="warmps", bufs=1, space="PSUM") as warmps,
        ):
            wsrc = warmp.tile([128, 512], f16, name="warm_src")
            nc.vector.memset(wsrc[:], 0.0)
            wps = warmps.tile([128, 512], f32, name="warm_ps")
            for _ in range(20):
                nc.tensor.matmul(
                    wps[:], wsrc[:, 0:128], wsrc[:],
                    start=True, stop=True, skip_group_check=True,
                )

        def qk_half(wt, dst, xo):
            """One 512-token half of a q/k projection (6 MMs + copy)."""
            ps = qkps.tile([128, 512], f32, tag="qk", name="ps_qk")
            for k in range(6):
                nc.tensor.matmul(
                    ps[:], wt[:, k, :], xTk[k][:, xo : xo + 512],
                    start=(k == 0), stop=(k == 5),
                )
            nc.vector.tensor_copy(dst[:, xo : xo + 512], ps[:])

        def emit_qk_mms(state, half):
            wqh, wkh, qh, kh = state
            qk_half(wqh, qh, half * 512)
            qk_half(wkh, kh, half * 512)

        # head 0's projection, both halves, right away
        emit_qk_mms(st0, 0)
        emit_qk_mms(st0, 1)

        ep = attn.enter_context(tc.tile_pool(name="estrip", bufs=3))
        fin = attn.enter_context(tc.tile_pool(name="fin", bufs=4))
        tstage = attn.enter_context(tc.tile_pool(name="tstage", bufs=2))
        spool = attn.enter_context(tc.tile_pool(name="spool", bufs=2, space="PSUM"))
        avps = None  # opened after the V projection's pool closes

        def emit_score_m(qh, kh, r, e12, m):
            """Scores+exp for one m-chunk of a 512-token strip. The two
            chain matmuls use disjoint PE row groups and disjoint PSUM
            banks, so the pair streams as one 512-col pass; one exp
            covers both chains."""
            nsl = slice(r * 512, (r + 1) * 512)
            msl = slice(m * 128, (m + 1) * 128)
            sp = spool.tile([128, 2, 512], f32, tag="s")
            nc.tensor.matmul(
                sp[:, 0, :], kh[0:64, msl], qh[0:64, nsl],
                start=True, stop=True,
            )
            nc.tensor.matmul(
                sp[:, 1, :], kh[64:128, msl], qh[64:128, nsl],
                start=True, stop=True,
            )
            nc.scalar.activation(
                e12[:, m, :].rearrange("p (a b) -> p a b", a=2),
                sp[:],
                AF.Exp,
                scale=SCALE,
            )

        def do_av_chunk(h, r, e12, c2):
                jn = r * 4 + c2
                o = avps.tile([128, 258], f32, tag="o", name="o_av")
                # Both accumulation chains share one PSUM bank. start=True
                # clears has_written bank-wide, so only the very first
                # matmul may set it; the second chain's first write still
                # overwrites because its bits are already clear. Chains
                # un-interleaved so consecutive LDW/MM pairs pipeline.
                for m in range(8):
                    nc.tensor.matmul(
                        o[:, 0:129],
                        e12[:, m, c2 * 128 : (c2 + 1) * 128],
                        v_aug[:, m, h, :],
                        start=(m == 0),
                        stop=(m == 7),
                        skip_group_check=True,
                    )
                for m in range(8):
                    nc.tensor.matmul(
                        o[:, 129:258],
                        e12[:, m, 512 + c2 * 128 : 512 + (c2 + 1) * 128],
                        v_aug[:, m, h, :],
                        start=False,
                        stop=(m == 7),
                        skip_group_check=True,
                    )
                # combine + LN stats (VectorE: PSUM port). One strided
                # reciprocal covers both denominators (cols 128 and 257).
                r12 = fin.tile([128, 2], f32, tag="r12")
                nc.vector.reciprocal(
                    r12[:], o[:].rearrange("p (c x) -> p c x", c=2)[:, :, 128:129]
                )
                t2 = fin.tile([128, 128], f32, tag="t2")
                nc.vector.tensor_scalar(
                    t2[:], o[:, 129:257], r12[:, 1:2], float(lam),
                    op0=OP.mult, op1=OP.mult,
                )
                nc.vector.scalar_tensor_tensor(
                    o_ln[:, jn, h, :],
                    o[:, 0:128],
                    r12[:, 0:1],
                    t2[:],
                    op0=OP.mult,
                    op1=OP.subtract,
                )
                st6 = fin.tile([128, 6], f32, tag="st6")
                nc.vector.bn_stats(st6[:], o_ln[:, jn, h, :])
                nc.vector.bn_aggr(stats_sb[:, h, jn, :], st6[:])

        def do_av(h, r, e12):
            for c2 in range(4):
                do_av_chunk(h, r, e12, c2)

        def head_tail(h):
            """rsqrt, LN apply, DMA-xbar transposes into a staging tile,
            then ONE GpSimd copy into o_lnT. The copy launders the 8
            DMA-queue semaphores into a single engine semaphore: without
            it every final-projection matmul re-waits on the DMA sems
            (~160ns each, measured). It runs on GpSimd because that queue
            is otherwise empty - it can block on the Sync engine's slow
            xbar transposes (~1.2us each, measured) without stalling
            anything."""
            # rs = exp(-0.5 * ln(var + eps)); Exp and Ln share one pinned
            # table set, so no reload happens here.
            nc.scalar.activation(
                sdbuf[:, h, :], stats_sb[:, h, :, 1], AF.Ln, bias=eps_sb[:],
            )
            nc.scalar.activation(
                rsbuf[:, h, :], sdbuf[:, h, :], AF.Exp, scale=-0.5,
            )
            last = h == 11
            stg = None if last else tstage.tile([128, 1024], f16, tag="tstage")
            for jn in range(8):
                nc.vector.tensor_scalar(
                    o_ln[:, jn, h, :],
                    o_ln[:, jn, h, :],
                    stats_sb[:, h, jn, 0:1],
                    rsbuf[:, h, jn : jn + 1],
                    op0=OP.subtract,
                    op1=OP.mult,
                )
                # head 11 is on the final-projection critical path: write
                # o_lnT directly, skipping the staging+copy latency (its
                # k=11 matmuls eat the small per-MM DMA-sem wait instead).
                dst = o_lnT[:, h, :] if last else stg[:]
                nc.sync.dma_start(
                    out=dst[:, jn * 128 : (jn + 1) * 128],
                    in_=o_ln[:, jn, h, :],
                    transpose=True,
                )
            if not last:
                nc.gpsimd.tensor_copy(o_lnT[:, h, :], stg[:])

        # ---- software-pipelined head loop ----
        # Per head h (steady state), the PE emission order is
        #   block A (strip r0):
        #     sc mp0, sc mp1, qk(h+1) q-half0, sc mp2, qk k-half0, sc mp3,
        #     AV(h-1, r1) c0..c3, tail(h-1)
        #   block B (strip r1):
        #     sc mp0, AV(h,r0) c0, sc mp1, AV c1, sc mp2, AV c2, sc mp3,
        #     AV c3, qk q-half1, qk k-half1
        # chosen so that each score pair's gate (the exp freeing its PSUM
        # tile, 2 tiles back) has completed by the time the PE reaches it.
        # h==0 nests the V projection (and its scoped PSUM pool) in place
        # of the AV work it doesn't have yet.
        states = {0: st0, 1: st1}
        pend = None  # (h, e12 of strip r1) awaiting AV + tail
        for h in range(12):
            qh, kh = states[h][2], states[h][3]
            stn = None
            if h + 1 < 12:
                stn = states[h + 1]
            if h + 2 < 12:
                states[h + 2] = emit_qk_dma(h + 2)
            states.pop(h - 1, None)
            if h == 9:
                # prefetch final-proj weights over the last heads' window
                for kk in range(12):
                    wpk = wpsp.tile([128, 768], f16, tag="wp", name=f"wpk{kk}")
                    nc.sync.dma_start(out=wpk[:], in_=WPR[kk])
                    wpks.append(wpk)

            # ---- block A (strip r0) ----
            e12_r0 = ep.tile([128, 8, 1024], f16, tag="e")
            for m in range(4):
                emit_score_m(qh, kh, 0, e12_r0, m)
            if stn is not None:
                qk_half(stn[0], stn[2], 0)
            for m in range(4, 8):
                emit_score_m(qh, kh, 0, e12_r0, m)
            if stn is not None:
                qk_half(stn[1], stn[3], 0)

            if h == 0:
                # ---- V projection, nested in head 0's slot ----
                with tc.tile_pool(name="vps", bufs=2, space="PSUM") as vps:
                    for t in range(8):
                        for cr in range(3):
                            ps = vps.tile([128, 512], f32, tag="v", name="vps")
                            for k in range(6):
                                nc.tensor.matmul(
                                    ps[:],
                                    xTk[k][:, t * 128 : (t + 1) * 128],
                                    wvk[k][:, cr * 512 : (cr + 1) * 512],
                                    start=(k == 0),
                                    stop=(k == 5),
                                )
                            nc.vector.tensor_copy(
                                v_aug[:, t, 4 * cr : 4 * cr + 4, 0:128],
                                ps[:].rearrange("p (h c) -> p h c", c=128),
                            )
                avps = attn.enter_context(
                    tc.tile_pool(name="avps", bufs=2, space="PSUM")
                )
            else:
                do_av(pend[0], 1, pend[1])
                head_tail(pend[0])

            # ---- block B (strip r1) ----
            e12_r1 = ep.tile([128, 8, 1024], f16, tag="e")
            for c2 in range(4):
                emit_score_m(qh, kh, 1, e12_r1, 2 * c2)
                emit_score_m(qh, kh, 1, e12_r1, 2 * c2 + 1)
                do_av_chunk(h, 0, e12_r0, c2)
            if stn is not None:
                qk_half(stn[0], stn[2], 512)
                qk_half(stn[1], stn[3], 512)
            pend = (h, e12_r1)

        do_av(pend[0], 1, pend[1])
        head_tail(pend[0])
        attn.close()  # frees qkps/spool/avps PSUM + qk/e12 SBUF

        # ---- Phase 3: final projection ----
        # mc-outer / k-inner: each 128-wide output-channel block finishes
        # (bias + DMA) while the next block computes, so only the last
        # block's drain is exposed. Stationary is reused across the two
        # token halves so walrus elides every second weight load.
        with (
            tc.tile_pool(name="tail", bufs=1) as tailp,
            tc.tile_pool(name="fps", bufs=3, space="PSUM") as fps,
        ):
            fout = tailp.tile([128, 6, 1024], f32)

            def mc_mms(mc, ks, fs):
                for k in ks:
                    wpk = wpks[k]
                    for nr2 in range(2):
                        nc.tensor.matmul(
                            fs[nr2][:],
                            wpk[:, mc * 128 : (mc + 1) * 128],
                            o_lnT[:, k, nr2 * 512 : (nr2 + 1) * 512],
                            start=(k == 0),
                            stop=(k == 11),
                        )

            def mc_drain(mc, fs):
                for nr2 in range(2):
                    nsl2 = slice(nr2 * 512, (nr2 + 1) * 512)
                    nc.vector.tensor_scalar(
                        fout[:, mc, nsl2],
                        fs[nr2][:],
                        bpp_sb[:, mc : mc + 1],
                        None,
                        op0=OP.add,
                    )
                # one issue per mc: descriptor generation on Sync is the
                # tail bottleneck, and one dma_start's descriptors already
                # spread over all 16 queues. The last block splits in two
                # so its final bytes ride partly-drained queues.
                if mc == 5:
                    nc.sync.dma_start(
                        out=OUT[:, mc, 0:512], in_=fout[:, mc, 0:512]
                    )
                    nc.sync.dma_start(
                        out=OUT[:, mc, 512:1024], in_=fout[:, mc, 512:1024]
                    )
                else:
                    nc.sync.dma_start(out=OUT[:, mc], in_=fout[:, mc])

            def fs_tiles():
                return [
                    fps.tile([128, 512], f32, tag=f"f{nr2}", name=f"fps{nr2}")
                    for nr2 in range(2)
                ]

            # The first three blocks defer their k=11 contribution: head
            # 11's transposes are still landing when phase 3 starts, and
            # ~66 k<11 matmuls are enough PE work to cover that latency
            # without a (clock-dropping) stall.
            fs012 = [fs_tiles() for _ in range(3)]
            for mc in range(3):
                mc_mms(mc, range(11), fs012[mc])
            for mc in range(3):
                mc_mms(mc, [11], fs012[mc])
                mc_drain(mc, fs012[mc])
            for mc in range(3, 5):
                fs = fs_tiles()
                mc_mms(mc, range(12), fs)
                mc_drain(mc, fs)
            # last block: finish the first token-half early so its bias+DMA
            # drain overlaps the second half's matmuls instead of the tail
            fs = fs_tiles()
            for k in range(12):
                nc.tensor.matmul(
                    fs[0][:], wpks[k][:, 5 * 128 : 6 * 128],
                    o_lnT[:, k, 0:512], start=(k == 0), stop=(k == 11),
                )
            nc.vector.tensor_scalar(
                fout[:, 5, 0:512], fs[0][:], bpp_sb[:, 5:6], None, op0=OP.add,
            )
            nc.sync.dma_start(out=OUT[:, 5, 0:512], in_=fout[:, 5, 0:512])
            for k in range(12):
                nc.tensor.matmul(
                    fs[1][:], wpks[k][:, 5 * 128 : 6 * 128],
                    o_lnT[:, k, 512:1024], start=(k == 0), stop=(k == 11),
                )
            nc.vector.tensor_scalar(
                fout[:, 5, 512:1024], fs[1][:], bpp_sb[:, 5:6], None,
                op0=OP.add,
            )
            nc.sync.dma_start(out=OUT[:, 5, 512:1024], in_=fout[:, 5, 512:1024])

    nc.compile()
    return nc


def _host_prep(x, Wq, Wk, Wv, gamma, beta, Wp, bp):
    x = np.ascontiguousarray(np.asarray(x, np.float32))
    Wq = np.asarray(Wq, np.float32)
    Wk = np.asarray(Wk, np.float32)
    Wv = np.asarray(Wv, np.float32)
    Wp = np.asarray(Wp, np.float32)
    bp = np.asarray(bp, np.float32)
    gamma = np.asarray(gamma, np.float32)
    beta = np.asarray(beta, np.float32)

    # xT per batch: [128, 6, 1024] with [p, k, n] = x[b, n, k*128+p]
    xTr = np.ascontiguousarray(
        x.transpose(0, 2, 1).reshape(B, 6, 128, N).transpose(0, 2, 1, 3)
    ).astype(np.float16)

    # W[qk]R: [12, 128, 6, 128] with [h, p, k, c] = W[k*128+p, h*128+c]
    def wqk_r(W):
        return np.ascontiguousarray(
            W.reshape(6, 128, 12, 128).transpose(2, 1, 0, 3)
        )

    WqR = wqk_r(Wq).astype(np.float16)
    WkR = wqk_r(Wk).astype(np.float16)
    # WvR: [128, 6, 1536] with [p, k, c] = Wv[k*128+p, c]
    WvR = np.ascontiguousarray(
        Wv.reshape(6, 128, 2 * C).transpose(1, 0, 2)
    ).astype(np.float16)
    # Fold gamma and the (1 - lambda_init) scale into Wp; beta into the bias.
    gfull = np.tile(gamma, H)  # [1536]
    Wpg = Wp * (OUT_SCALE * gfull)[:, None]
    bpp = bp + OUT_SCALE * (np.tile(beta, H) @ Wp)
    WpR = np.ascontiguousarray(Wpg.reshape(12, 128, C)).astype(np.float16)
    bppR = np.ascontiguousarray(bpp.reshape(6, 128).T)  # [128, 6]
    return xTr, WqR, WkR, WvR, WpR, bppR


def kernel(x, Wq, Wk, Wv, lam, gamma, beta, Wp, bp):
    global LAST_EXEC_NS
    import os

    from concourse.bass_utils import run_bass_kernel_spmd

    lam_f = float(np.asarray(lam))
    xTr, WqR, WkR, WvR, WpR, bppR = _host_prep(
        x, Wq, Wk, Wv, gamma, beta, Wp, bp
    )

    key = lam_f
    if key not in _BUILD_CACHE:
        _BUILD_CACHE[key] = _build(lam_f)
    nc = _BUILD_CACHE[key]

    in_maps = [
        {
            "xT": xTr[b],
            "WqR": WqR,
            "WkR": WkR,
            "WvR": WvR,
            "WpR": WpR,
            "bpp": bppR,
        }
        for b in range(B)
    ]

    trace = bool(os.environ.get("BASS_KERNEL_TRACE"))
    if trace:
        from concourse import bass_utils as _bu

        _bu.upload_artifacts = lambda tmpdir: "local://" + tmpdir
    res = run_bass_kernel_spmd(
        nc, in_maps, list(range(B)), trace=trace,
        **({"trace_cores": list(range(B))} if trace else {}),
    )
    LAST_EXEC_NS = res.exec_time_ns

    out = np.empty((B, N, C), np.float32)
    for b in range(B):
        outT = res.results[b]["outT"]  # [128, 6, 1024]
        out[b] = outT.transpose(2, 1, 0).reshape(N, C)
    return out
